# revision 24
# baseline (speedup 1.0000x reference)
"""ABC-Conv (binary conv, 3 estimators) on 8 trn2 NeuronCores — fp8 DoubleRow,
transposed-output layout.

Math: reference computes
    xq   = sign(x)
    beta = boxfilter3x3(sum_c |x|) / 1152                [B,110,110]
    out  = sum_e conv(xq, sign(kernels[e])) * beta[...,None] * alphas[e]

conv is linear in its kernel, so the estimator loop folds into ONE conv with
W = sum_e sign(kernels[e]) * alphas[e].  W is quantized to fp8 e4m3 with a
per-output-channel scale s[f] (grid search); the conv runs as fp8 DoubleRow
matmuls (2 taps contracted per pass).

Transposed layout: stationary = xq pixel-tile pair planes [cin, 2tap, 128px],
moving = w8 [cin, 2tap, 256f], psum [128px, 2tile, 256f] (one 10-matmul chain
fills a full psum bank; the first start=True zeroes the bank).  The output
PIXEL lives in the partition dim, so the per-pixel beta scale is a
per-partition scalar: the drain is a single multiply that can run as a DVE
tensor_tensor over a whole bank (two beta columns broadcast) or a ScalarE
activation per tile — no betab broadcast pipeline at all.

Scheduling notes (learned from traces):
  - tile deps are whole-tile: all big buffers are split PER IMAGE so img0's
    work never waits on img1 writes.
  - the x_pm -> s -> betaCol pipeline has no other deps: x_pm DMAs are
    front-loaded into persistent buffers (pieces of 14/35/49 s-cols) and the
    28 DVE reduce slices are sprinkled early between drains, so the last
    output tiles are not gated on the last input chunk.
  - DMA issue costs ~0.6-0.9us of the issuing engine's queue: xT on sync,
    stores on sync, x_pm + xqT-alias copies on gpsimd, w8 on scalar, boxm on
    vector — the first xT chunk is the only thing ahead of the conv start.
  - drain engines alternate [DVE, DVE, ScalarE] per psum bank: DVE drains a
    whole bank (tensor_mul, beta pair broadcast), ScalarE per tile
    (activation, per-partition scale); ScalarE also runs sign + betaCol.

Sharding: data-parallel over batch, 2 images per core, weights replicated.
Host applies sinv[f] (fp8 scale inverse) and slices the garbage columns
(w in {110,111}) and rows (>= 110) during the gather.
"""

import sys

sys.path.insert(0, "/opt/trn_rl_repo")

import bass_rust
import ml_dtypes
import numpy as np

import concourse.tile as tile
from concourse import bacc, mybir
from concourse.bass import ds
from concourse.bass_utils import run_bass_kernel_spmd

F32 = mybir.dt.float32
F16 = mybir.dt.float16
BF16 = mybir.dt.bfloat16
FP8 = mybir.dt.float8e4

N_CORES = 8
B_PER_CORE = 2
H = W_IMG = 112
CIN = 128
F = 256
E = 3
D_DIM = 9 * CIN  # 1152

IMG_PIX = H * W_IMG          # 12544
IMG_PAD = (H + 2) * W_IMG    # 12768 (2 zero rows terminate window reads)
OUT_ROWS = 110
OUT_PIX = OUT_ROWS * W_IMG   # 12320
N_TILES = 97                 # 128-px output tiles per image (96 full + tail)
STAGE_PIX = N_TILES * 128    # 12416 staged px per img (tail padded)
S_SEG = 100                  # s columns per img (98 real + 2 zero pad)
S_TILES = 98

# DoubleRow plane strides must be %16==0 (HW constraint, probed).  A +1-px
# shifted alias of xqT (xqT2, built by sbuf->sbuf DMA at offset XQ2 inside the
# per-image buffer) makes (d, d+1) pairs legal; (2,114) pairs across rows
# (stride 112); tap (2,2) is a stride-0 self-pair carrying a two-term fp8
# expansion (Wa=q(sW), Wb=q(sW-Wa)).
XQ2 = IMG_PAD  # 12768, %16==0 (alias offset within the per-image xq buffer)
PAIR_OFF = [(0, XQ2), (2, 114), (112, XQ2 + 112), (224, XQ2 + 224), (226, 226)]
NPAIR = 5
# w8 slot order (2i, 2i+1) = the tap (kh,kw) each plane multiplies
SLOT_TAPS = [(0, 0), (0, 1), (0, 2), (1, 2), (1, 0), (1, 1), (2, 0), (2, 1)]

N_PAIRS = 49                 # psum banks per image (48 full pairs + tail)
STG = 4                      # px-tiles per output store DMA
# betaCol segments, aligned so each seg's s-window (t0..t0+cn+2) stays within
# the x_pm pieces already landed: piece0 (cols<14) serves the first two segs
BSEGS = [(0, 8), (8, 3), (11, 8), (19, 8), (27, 8), (35, 8), (43, 3),
         (46, 8), (54, 8), (62, 5), (67, 8), (75, 8), (83, 8), (91, 6)]
RSL = 7                      # s-cols per reduce slice (14 slices per image)
N_RSL = 14

XT_CH = [(0, 512), (512, 512), (1024, 1024), (2048, 2048), (4096, 2048),
         (6144, 2048), (8192, 2048), (10240, 2528)]  # covers IMG_PAD
XPM_PC = [(0, 14), (14, 35), (49, 21), (70, 28)]  # front-loaded x_pm pieces


def _box_matrices():
    """beta_pre[p, t] = sum_q sum_k Mq[k,p] * s[k, t+q]; window offsets reach
    p+353, spanning three 128-columns of s."""
    doff = [kh * W_IMG + kw for kh in range(3) for kw in range(3)]
    ms = np.zeros((3, 128, 128), np.float32)
    for p in range(128):
        for d in doff:
            k = p + d
            ms[k // 128, k % 128, p] = 1.0
    return ms.astype(np.float16)


def _pair_view(ap_full, da, db, n):
    """[128, 2, n] AP over a [128, X] sbuf tile: planes at free offsets da, db."""
    dim0 = list(ap_full.ap[0])
    return bass_rust.AP(
        ap_full.tensor, ap_full.offset + da, [dim0, [db - da, 2], [1, n]]
    )


def build_nc():
    nc = bacc.Bacc("TRN2", target_bir_lowering=False, debug=False)
    xt_d = nc.dram_tensor("xT", [CIN, B_PER_CORE * IMG_PAD], BF16, kind="ExternalInput").ap()
    xpm_d = nc.dram_tensor("xpm", [128, B_PER_CORE * S_TILES, CIN], BF16, kind="ExternalInput").ap()
    w_d = nc.dram_tensor("w8", [CIN, 2 * NPAIR * F], FP8, kind="ExternalInput").ap()
    m_d = nc.dram_tensor("boxm", [3, 128, 128], F16, kind="ExternalInput").ap()
    o_d = nc.dram_tensor("out", [B_PER_CORE * N_TILES * 128, F], BF16, kind="ExternalOutput").ap()

    with tile.TileContext(nc) as tc:
        with (
            tc.tile_pool(name="const", bufs=1) as constp,
            tc.tile_pool(name="big", bufs=1) as bigp,
            tc.tile_pool(name="xin", bufs=3) as xinp,
            tc.tile_pool(name="o2", bufs=3) as o2p,
            tc.tile_pool(name="psum", bufs=6, space="PSUM") as psump,
            tc.tile_pool(name="psb", bufs=2, space="PSUM") as psbp,
        ):
            # ---------- constants ----------
            boxm = constp.tile([128, 3, 128], F16)
            nc.scalar.dma_start(boxm[:, :, :], m_d.rearrange("m k p -> k m p"))
            w8 = constp.tile([128, 2 * NPAIR, F], FP8)
            nc.scalar.dma_start(w8[:, :, :], w_d.rearrange("c (t f) -> c t f", t=2 * NPAIR))

            # ---------- per-image persistent buffers ----------
            xq = [
                bigp.tile([128, 2 * IMG_PAD], FP8, name=f"xq{b}")
                for b in range(B_PER_CORE)
            ]
            s16 = [
                bigp.tile([128, S_SEG], F16, name=f"s16_{b}")
                for b in range(B_PER_CORE)
            ]
            betaCol = [
                bigp.tile([128, N_TILES], F32, name=f"betaCol{b}")
                for b in range(B_PER_CORE)
            ]
            xpmt = [[
                bigp.tile([128, cn, CIN], BF16, name=f"xpm{b}_{c0}")
                for c0, cn in XPM_PC
            ] for b in range(B_PER_CORE)]
            for b in range(B_PER_CORE):
                nc.vector.memset(s16[b][:, ds(S_TILES, 2)], 0.0)

            # front-loaded x_pm DMAs are emitted just after the first xT
            # chunk (see pipeline below), split across the scalar and sync
            # queues; gpsimd stays clean so the xq alias copies (conv
            # critical path) never queue behind these bulk transfers
            # p0 pieces ride gpsimd: its queue is empty before the first
            # alias, so the ~2.3us-per-DMA cold DGE overhead lands once, not
            # third-in-line behind w8+boxm — s data on chip by ~9.5us
            xpm_q = {
                (0, 0): nc.gpsimd, (1, 0): nc.gpsimd,
                (0, 1): nc.sync, (1, 1): nc.sync,
                (0, 2): nc.scalar, (1, 2): nc.scalar,
                (0, 3): nc.scalar, (1, 3): nc.scalar,
            }

            def emit_xpm_pieces():
                for pc, (c0, cn) in enumerate(XPM_PC):
                    for b in range(B_PER_CORE):
                        xpm_q[(b, pc)].dma_start(
                            xpmt[b][pc][:, :, :],
                            xpm_d[:, ds(b * S_TILES + c0, cn), :],
                        )

            shift_pos = [0] * B_PER_CORE
            nbank = 0
            nstore = 0

            def emit_xt_chunk(b, c0, npix):
                xst = xinp.tile([128, 2560], BF16, tag="xtst")
                nc.sync.dma_start(
                    xst[:, :npix], xt_d[:, ds(b * IMG_PAD + c0, npix)]
                )
                nc.scalar.sign(xq[b][:, ds(c0, npix)], xst[:, :npix])
                # extend the +1-shifted alias (xqT2) as far as signed data allows
                new_end = c0 + npix - 1
                ln = new_end - shift_pos[b]
                nc.gpsimd.dma_start(
                    xq[b][:, ds(XQ2 + shift_pos[b], ln)],
                    xq[b][:, ds(shift_pos[b] + 1, ln)],
                )
                shift_pos[b] = new_end

            def emit_reduce_slice(b, j):
                # s-cols [7j, 7j+7) from whichever x_pm piece holds them
                col = RSL * j
                for pc, (c0, cn) in enumerate(XPM_PC):
                    if c0 <= col < c0 + cn:
                        break
                with nc.allow_low_precision("s16 f16 channel sums (beta ~1e-3)"):
                    nc.vector.tensor_reduce(
                        s16[b][:, ds(col, RSL)],
                        xpmt[b][pc][:, ds(col - c0, RSL), :],
                        axis=mybir.AxisListType.X,
                        op=mybir.AluOpType.add,
                        apply_absolute_value=True,
                    )

            def emit_beta_seg(b, t0, cn):
                # box filter with boxm stationary: psum comes out [pos, tile]
                btp = psbp.tile([128, 8], F32, tag="btp")
                for q in range(3):
                    nc.tensor.matmul(
                        btp[:, :cn],
                        lhsT=boxm[:, q, :],
                        rhs=s16[b][:, ds(t0 + q, cn)],
                        start=(q == 0),
                        stop=(q == 2),
                    )
                # scale on DVE: the scalar queue's sign chunks would delay
                # this, and the single psb bank WARs the next seg's matmuls
                # against it
                nc.vector.tensor_scalar_mul(
                    betaCol[b][:, ds(t0, cn)], btp[:, :cn], 1.0 / D_DIM
                )

            def emit_mm_pair(b, p):
                ntile = 2 if p < N_PAIRS - 1 else 1
                ps = psump.tile([128, 2, F], F32, tag="ps", name=f"ps{b}_{p}")
                base = (2 * p) * 128
                nmm = NPAIR * ntile
                k = 0
                for j in range(ntile):
                    for i, (da, db) in enumerate(PAIR_OFF):
                        nc.tensor.matmul(
                            ps[:, j, :],
                            lhsT=_pair_view(
                                xq[b][:, :], base + j * 128 + da, base + j * 128 + db, 128
                            ),
                            rhs=w8[:, ds(2 * i, 2), :],
                            start=(k == 0),
                            stop=(k == nmm - 1),
                            perf_mode=mybir.MatmulPerfMode.DoubleRow,
                        )
                        k += 1
                return ps

            o2cur = [None] * B_PER_CORE

            def emit_drain_bank(b, p, ps_tiles):
                nonlocal nbank, nstore
                t0 = 2 * p
                ntile = 2 if p < N_PAIRS - 1 else 1
                ps = ps_tiles[p]
                if t0 % STG == 0:
                    o2cur[b] = o2p.tile(
                        [128, STG, F], BF16, tag="o2", name=f"o2_{b}_{t0}"
                    )
                o2t = o2cur[b]
                sl = t0 % STG
                if p >= N_PAIRS - 4:
                    # tail banks: split tiles across both engines so the
                    # post-PE drain backlog clears in parallel
                    for j in range(ntile):
                        eng = nc.vector if (t0 + j) % 2 == 0 else None
                        if eng is not None:
                            nc.vector.tensor_scalar_mul(
                                o2t[:, sl + j, :],
                                ps[:, j, :],
                                betaCol[b][:, ds(t0 + j, 1)],
                            )
                        else:
                            nc.scalar.mul(
                                o2t[:, sl + j, :],
                                ps[:, j, :],
                                betaCol[b][:, ds(t0 + j, 1)],
                            )
                elif ntile == 2 and nbank % 3 != 2:
                    bc = (
                        betaCol[b][:, ds(t0, 2)]
                        .unsqueeze(2)
                        .broadcast_to([128, 2, F])
                    )
                    nc.vector.tensor_mul(o2t[:, ds(sl, 2), :], ps[:, :, :], bc)
                else:
                    for j in range(ntile):
                        nc.scalar.mul(
                            o2t[:, sl + j, :],
                            ps[:, j, :],
                            betaCol[b][:, ds(t0 + j, 1)],
                        )
                nbank += 1
                t_last = t0 + ntile - 1
                if t_last % STG == STG - 1 or t_last == N_TILES - 1:
                    n = t_last % STG + 1
                    g0 = t_last - n + 1
                    nc.sync.dma_start(
                        o_d[ds((b * N_TILES + g0) * 128, n * 128), :].rearrange(
                            "(t p) f -> p t f", t=n
                        ),
                        o2t[:, :n, :],
                    )
                    nstore += 1

            # ---------- interleaved two-image pipeline ----------
            ps_tiles = [[] for _ in range(B_PER_CORE)]
            rsl = [0] * B_PER_CORE      # reduce slices emitted
            bq = [0] * B_PER_CORE       # next beta seg
            betac = [0] * B_PER_CORE    # tiles covered by betaCol
            mm_p = [0] * B_PER_CORE     # next psum bank (px-tile pair)
            dr_p = [0] * B_PER_CORE     # next bank to drain
            pcov_v = [0] * B_PER_CORE
            for c in range(len(XT_CH)):
                for b in range(B_PER_CORE):
                    c0, npix = XT_CH[c]
                    emit_xt_chunk(b, c0, npix)
                    pcov_v[b] = c0 + npix
                if c == 0:
                    emit_xpm_pieces()
                progressed = True
                while progressed:
                    progressed = False
                    for b in range(B_PER_CORE):
                        # demand-driven beta: stay ~5 banks ahead of drains,
                        # pulling reduce slices just-in-time so the DVE queue
                        # never front-loads reduces ahead of pending drains
                        while bq[b] < len(BSEGS) and betac[b] < min(
                            2 * dr_p[b] + 14, N_TILES
                        ):
                            t0, cn = BSEGS[bq[b]]
                            need = t0 + cn + 3
                            while (
                                rsl[b] < N_RSL
                                and RSL * rsl[b] + 2 * (rsl[b] == N_RSL) < need
                            ):
                                emit_reduce_slice(b, rsl[b])
                                rsl[b] += 1
                            emit_beta_seg(b, t0, cn)
                            betac[b] = t0 + cn
                            bq[b] += 1
                            progressed = True
                        # mid-kernel: drip remaining reduce slices so late
                        # beta segs never wait on them (data resident by now)
                        if rsl[b] < N_RSL and dr_p[b] >= 8:
                            emit_reduce_slice(b, rsl[b])
                            rsl[b] += 1
                            progressed = True
                        if mm_p[b] < N_PAIRS:
                            p = mm_p[b]
                            last_t = min(2 * p + 1, N_TILES - 1)
                            # planes reach px base+353 (max tap offset 226+127)
                            need_px = 128 * last_t + 354
                            if need_px <= pcov_v[b]:
                                ps_tiles[b].append(emit_mm_pair(b, p))
                                mm_p[b] += 1
                                progressed = True
                        while dr_p[b] < mm_p[b] and min(
                            2 * dr_p[b] + 2, N_TILES
                        ) <= betac[b]:
                            emit_drain_bank(b, dr_p[b], ps_tiles[b])
                            dr_p[b] += 1
                            progressed = True
            assert rsl == [N_RSL] * B_PER_CORE, rsl
            assert bq == [len(BSEGS)] * B_PER_CORE, bq
            assert mm_p == [N_PAIRS] * B_PER_CORE, mm_p
            assert dr_p == [N_PAIRS] * B_PER_CORE, dr_p

    nc.compile()
    return nc


_NC = None


def _get_nc():
    global _NC
    if _NC is None:
        _NC = build_nc()
    return _NC


def _quantize_weights(kernels, alphas):
    """Fold estimators, then per-channel-scale fp8 e4m3 quantization."""
    sgn = np.where(kernels >= 0, 1.0, -1.0).astype(np.float32)  # [E,3,3,128,256]
    W = np.einsum("ehwcf,ef->hwcf", sgn, alphas.astype(np.float32))  # [3,3,128,256]
    # scale search on the single-term taps; tap (2,2) is two-term (near exact)
    Wf = np.stack([W[kh, kw] for kh, kw in SLOT_TAPS]).reshape(8 * CIN, F)
    scales = np.geomspace(6.0, 100.0, 385).astype(np.float32)
    q = (Wf[None, :, :] * scales[:, None, None]).astype(ml_dtypes.float8_e4m3fn)
    err = ((q.astype(np.float32) / scales[:, None, None] - Wf[None]) ** 2).sum(axis=1)
    s = scales[np.argmin(err, axis=0)]  # [F]
    Wq = (W * s).astype(ml_dtypes.float8_e4m3fn)  # [3,3,128,256]
    Wres = W * s - Wq.astype(np.float32)
    Wq2 = Wres.astype(ml_dtypes.float8_e4m3fn)  # second term for tap (2,2)
    # slot pairs (2i, 2i+1) follow PAIR_OFF via SLOT_TAPS; tap (2,2) two-term
    w8 = np.zeros((CIN, 2 * NPAIR, F), ml_dtypes.float8_e4m3fn)
    for j, (kh, kw) in enumerate(SLOT_TAPS):
        w8[:, j, :] = Wq[kh, kw]
    w8[:, 8, :] = Wq[2, 2]
    w8[:, 9, :] = Wq2[2, 2]
    sinv = (1.0 / s).astype(np.float32)  # [F]; applied on host after gather
    return np.ascontiguousarray(w8.reshape(CIN, 2 * NPAIR * F)), sinv


def _in_maps(x, kernels, alphas):
    x = np.asarray(x, np.float32)
    kernels = np.asarray(kernels, np.float32)
    alphas = np.asarray(alphas, np.float32)
    w8, sinv = _quantize_weights(kernels, alphas)
    boxm = _box_matrices()

    xb = x.astype(ml_dtypes.bfloat16)  # sign-exact; |x| sums lose <0.1%
    xs = xb.reshape(N_CORES, B_PER_CORE, IMG_PIX, CIN)
    maps = []
    for c in range(N_CORES):
        xT = np.zeros((CIN, B_PER_CORE * IMG_PAD), ml_dtypes.bfloat16)
        for b in range(B_PER_CORE):
            xT[:, b * IMG_PAD : b * IMG_PAD + IMG_PIX] = xs[c, b].T
        xpm = np.ascontiguousarray(
            xs[c].reshape(B_PER_CORE, S_TILES, 128, CIN)
            .transpose(2, 0, 1, 3)
            .reshape(128, B_PER_CORE * S_TILES, CIN)
        )
        maps.append(
            {
                "xT": np.ascontiguousarray(xT),
                "xpm": xpm,
                "w8": w8,
                "boxm": boxm,
            }
        )
    return maps, sinv


def _gather(results, sinv):
    outs = []
    sv = sinv.reshape(1, 1, 1, F)
    for c in range(N_CORES):
        o = np.asarray(results[c]["out"]).astype(np.float32)  # [24832, 256]
        o = o.reshape(B_PER_CORE, STAGE_PIX, F)[:, :OUT_PIX]
        o = o.reshape(B_PER_CORE, OUT_ROWS, W_IMG, F)[:, :, :OUT_ROWS] * sv
        outs.append(o)
    return np.ascontiguousarray(np.concatenate(outs, axis=0))


def kernel(x, kernels, alphas):
    nc = _get_nc()
    maps, sinv = _in_maps(x, kernels, alphas)
    res = run_bass_kernel_spmd(nc, maps, core_ids=list(range(N_CORES)))
    return _gather(res.results, sinv)


def _install_profile_hook():
    """The agent image's antenv lacks axon_hooks; recreate it so
    run_bass_kernel_spmd(trace=True) can NTFF-profile via libaxon_pjrt.so."""
    import types

    import antenv

    if "antenv.axon_hooks" in sys.modules:
        return
    mod = types.ModuleType("antenv.axon_hooks")
    holder = {}
    mod.set_axon_ntff_profile_hook = lambda h: holder.__setitem__("h", h)
    mod.get_axon_ntff_profile_hook = lambda: holder.get("h")
    sys.modules["antenv.axon_hooks"] = mod
    antenv.axon_hooks = mod

    from trn_agent_boot.trn_boot import _ntff_profile_via_ctypes

    hook = _ntff_profile_via_ctypes("/opt/axon/libaxon_pjrt.so")
    mod.set_axon_ntff_profile_hook(hook)

    # upload_artifacts wants a cloud bucket; keep everything local instead.
    import concourse.bass_utils as bu

    bu.upload_artifacts = lambda tmpdir: tmpdir


def run_profiled(x, kernels, alphas, tmpdir=None):
    """Returns (output, exec_time_ns, profile_json_path)."""
    _install_profile_hook()
    nc = _get_nc()
    maps, sinv = _in_maps(x, kernels, alphas)
    res = run_bass_kernel_spmd(
        nc,
        maps,
        core_ids=list(range(N_CORES)),
        trace=True,
        tmpdir=tmpdir,
    )
    return _gather(res.results, sinv), res.exec_time_ns, res.profile_json


# revision 26
# speedup vs baseline: 1.1791x; 1.1791x over previous
"""ABC-Conv (binary conv, 3 estimators) on 8 trn2 NeuronCores — fp8 DoubleRow,
transposed-output layout.

Math: reference computes
    xq   = sign(x)
    beta = boxfilter3x3(sum_c |x|) / 1152                [B,110,110]
    out  = sum_e conv(xq, sign(kernels[e])) * beta[...,None] * alphas[e]

conv is linear in its kernel, so the estimator loop folds into ONE conv with
W = sum_e sign(kernels[e]) * alphas[e].  W is quantized to fp8 e4m3 with a
per-output-channel scale s[f] (grid search); the conv runs as fp8 DoubleRow
matmuls (2 taps contracted per pass).

Transposed layout: stationary = xq pixel-tile pair planes [cin, 2tap, 128px],
moving = w8 [cin, 2tap, 256f], psum [128px, 2tile, 256f] (one 10-matmul chain
fills a full psum bank; the first start=True zeroes the bank).  The output
PIXEL lives in the partition dim, so the per-pixel beta scale is a
per-partition scalar: the drain is a single multiply that can run as a DVE
tensor_tensor over a whole bank (two beta columns broadcast) or a ScalarE
activation per tile — no betab broadcast pipeline at all.

Scheduling notes (learned from traces):
  - tile deps are whole-tile: all big buffers are split PER IMAGE so img0's
    work never waits on img1 writes.
  - the x_pm -> s -> betaCol pipeline has no other deps: x_pm DMAs are
    front-loaded into persistent buffers (pieces of 14/35/49 s-cols) and the
    28 DVE reduce slices are sprinkled early between drains, so the last
    output tiles are not gated on the last input chunk.
  - DMA issue costs ~0.6-0.9us of the issuing engine's queue: xT on sync,
    stores on sync, x_pm + xqT-alias copies on gpsimd, w8 on scalar, boxm on
    vector — the first xT chunk is the only thing ahead of the conv start.
  - drain engines alternate [DVE, DVE, ScalarE] per psum bank: DVE drains a
    whole bank (tensor_mul, beta pair broadcast), ScalarE per tile
    (activation, per-partition scale); ScalarE also runs sign + betaCol.

Sharding: data-parallel over batch, 2 images per core, weights replicated.
Host applies sinv[f] (fp8 scale inverse) and slices the garbage columns
(w in {110,111}) and rows (>= 110) during the gather.
"""

import sys

sys.path.insert(0, "/opt/trn_rl_repo")

import bass_rust
import ml_dtypes
import numpy as np

import concourse.tile as tile
from concourse import bacc, mybir
from concourse.bass import ds
from concourse.bass_utils import run_bass_kernel_spmd

F32 = mybir.dt.float32
F16 = mybir.dt.float16
BF16 = mybir.dt.bfloat16
FP8 = mybir.dt.float8e4

N_CORES = 8
B_PER_CORE = 2
H = W_IMG = 112
CIN = 128
F = 256
E = 3
D_DIM = 9 * CIN  # 1152

IMG_PIX = H * W_IMG          # 12544
IMG_PAD = (H + 2) * W_IMG    # 12768 (2 zero rows terminate window reads)
OUT_ROWS = 110
OUT_PIX = OUT_ROWS * W_IMG   # 12320
N_TILES = 97                 # 128-px output tiles per image (96 full + tail)
STAGE_PIX = N_TILES * 128    # 12416 staged px per img (tail padded)
S_SEG = 100                  # s columns per img (98 real + 2 zero pad)
S_TILES = 98

# DoubleRow plane strides must be %16==0 (HW constraint, probed).  A +1-px
# shifted alias of xqT (xqT2, built by sbuf->sbuf DMA at offset XQ2 inside the
# per-image buffer) makes (d, d+1) pairs legal; (2,114) pairs across rows
# (stride 112); tap (2,2) is a stride-0 self-pair carrying a two-term fp8
# expansion (Wa=q(sW), Wb=q(sW-Wa)).
XQ2 = IMG_PAD  # 12768, %16==0 (alias offset within the per-image xq buffer)
PAIR_OFF = [(0, XQ2), (2, 114), (112, XQ2 + 112), (224, XQ2 + 224), (226, 226)]
NPAIR = 5
# w8 slot order (2i, 2i+1) = the tap (kh,kw) each plane multiplies
SLOT_TAPS = [(0, 0), (0, 1), (0, 2), (1, 2), (1, 0), (1, 1), (2, 0), (2, 1)]

N_PAIRS = 49                 # psum banks per image (48 full pairs + tail)
STG = 4                      # px-tiles per output store DMA
# betaCol segments, aligned so each seg's s-window (t0..t0+cn+2) stays within
# the x_pm pieces already landed: piece0 (cols<14) serves the first two segs
BSEGS = [(0, 8), (8, 3), (11, 8), (19, 8), (27, 8), (35, 8), (43, 3),
         (46, 8), (54, 8), (62, 5), (67, 8), (75, 8), (83, 8), (91, 6)]
RSL = 7                      # s-cols per reduce slice (14 slices per image)
N_RSL = 14

XT_CH = [(0, 512), (512, 512), (1024, 1024), (2048, 2048), (4096, 2048),
         (6144, 2048), (8192, 2048), (10240, 2528)]  # covers IMG_PAD
XPM_PC = [(0, 14), (14, 35), (49, 21), (70, 28)]  # front-loaded x_pm pieces


def _box_matrices():
    """beta_pre[p, t] = sum_q sum_k Mq[k,p] * s[k, t+q]; window offsets reach
    p+353, spanning three 128-columns of s."""
    doff = [kh * W_IMG + kw for kh in range(3) for kw in range(3)]
    ms = np.zeros((3, 128, 128), np.float32)
    for p in range(128):
        for d in doff:
            k = p + d
            ms[k // 128, k % 128, p] = 1.0
    return ms.astype(np.float16)


def _pair_view(ap_full, da, db, n):
    """[128, 2, n] AP over a [128, X] sbuf tile: planes at free offsets da, db."""
    dim0 = list(ap_full.ap[0])
    return bass_rust.AP(
        ap_full.tensor, ap_full.offset + da, [dim0, [db - da, 2], [1, n]]
    )


def build_nc():
    nc = bacc.Bacc("TRN2", target_bir_lowering=False, debug=False)
    xt_d = nc.dram_tensor("xT", [CIN, B_PER_CORE * IMG_PAD], BF16, kind="ExternalInput").ap()
    xpm_d = nc.dram_tensor("xpm", [128, B_PER_CORE * S_TILES, CIN], BF16, kind="ExternalInput").ap()
    w_d = nc.dram_tensor("w8", [CIN, 2 * NPAIR * F], FP8, kind="ExternalInput").ap()
    m_d = nc.dram_tensor("boxm", [3, 128, 128], F16, kind="ExternalInput").ap()
    o_d = nc.dram_tensor("out", [B_PER_CORE * N_TILES * 128, F], BF16, kind="ExternalOutput").ap()

    with tile.TileContext(nc) as tc:
        with (
            tc.tile_pool(name="const", bufs=1) as constp,
            tc.tile_pool(name="big", bufs=1) as bigp,
            tc.tile_pool(name="xin", bufs=3) as xinp,
            tc.tile_pool(name="o2", bufs=3) as o2p,
            tc.tile_pool(name="psum", bufs=6, space="PSUM") as psump,
            tc.tile_pool(name="psb", bufs=2, space="PSUM") as psbp,
        ):
            # ---------- constants ----------
            boxm = constp.tile([128, 3, 128], F16)
            nc.scalar.dma_start(boxm[:, :, :], m_d.rearrange("m k p -> k m p"))
            w8 = constp.tile([128, 2 * NPAIR, F], FP8)
            nc.scalar.dma_start(w8[:, :, :], w_d.rearrange("c (t f) -> c t f", t=2 * NPAIR))

            # ---------- per-image persistent buffers ----------
            xq = [
                bigp.tile([128, 2 * IMG_PAD], FP8, name=f"xq{b}")
                for b in range(B_PER_CORE)
            ]
            s16 = [
                bigp.tile([128, S_SEG], F16, name=f"s16_{b}")
                for b in range(B_PER_CORE)
            ]
            betaCol = [
                bigp.tile([128, N_TILES], F32, name=f"betaCol{b}")
                for b in range(B_PER_CORE)
            ]
            xpmt = [[
                bigp.tile([128, cn, CIN], BF16, name=f"xpm{b}_{c0}")
                for c0, cn in XPM_PC
            ] for b in range(B_PER_CORE)]
            for b in range(B_PER_CORE):
                nc.vector.memset(s16[b][:, ds(S_TILES, 2)], 0.0)

            # front-loaded x_pm DMAs are emitted just after the first xT
            # chunk (see pipeline below), split across the scalar and sync
            # queues; gpsimd stays clean so the xq alias copies (conv
            # critical path) never queue behind these bulk transfers
            # gpsimd is reserved for the xq alias copies (conv critical path,
            # every chunk) — parking even two x_pm pieces there costs +22us
            xpm_q = {
                (0, 0): nc.scalar, (1, 0): nc.sync,
                (0, 1): nc.sync, (1, 1): nc.sync,
                (0, 2): nc.scalar, (1, 2): nc.scalar,
                (0, 3): nc.scalar, (1, 3): nc.scalar,
            }

            def emit_xpm_pieces():
                for pc, (c0, cn) in enumerate(XPM_PC):
                    for b in range(B_PER_CORE):
                        xpm_q[(b, pc)].dma_start(
                            xpmt[b][pc][:, :, :],
                            xpm_d[:, ds(b * S_TILES + c0, cn), :],
                        )

            shift_pos = [0] * B_PER_CORE
            nbank = 0
            nstore = 0

            def emit_xt_chunk(b, c0, npix):
                xst = xinp.tile([128, 2560], BF16, tag="xtst")
                nc.sync.dma_start(
                    xst[:, :npix], xt_d[:, ds(b * IMG_PAD + c0, npix)]
                )
                nc.scalar.sign(xq[b][:, ds(c0, npix)], xst[:, :npix])
                # extend the +1-shifted alias (xqT2) as far as signed data allows
                new_end = c0 + npix - 1
                ln = new_end - shift_pos[b]
                nc.gpsimd.dma_start(
                    xq[b][:, ds(XQ2 + shift_pos[b], ln)],
                    xq[b][:, ds(shift_pos[b] + 1, ln)],
                )
                shift_pos[b] = new_end

            def emit_reduce_slice(b, j):
                # s-cols [7j, 7j+7) from whichever x_pm piece holds them
                col = RSL * j
                for pc, (c0, cn) in enumerate(XPM_PC):
                    if c0 <= col < c0 + cn:
                        break
                with nc.allow_low_precision("s16 f16 channel sums (beta ~1e-3)"):
                    nc.vector.tensor_reduce(
                        s16[b][:, ds(col, RSL)],
                        xpmt[b][pc][:, ds(col - c0, RSL), :],
                        axis=mybir.AxisListType.X,
                        op=mybir.AluOpType.add,
                        apply_absolute_value=True,
                    )

            def emit_beta_seg(b, t0, cn):
                # box filter with boxm stationary: psum comes out [pos, tile]
                btp = psbp.tile([128, 8], F32, tag="btp")
                for q in range(3):
                    nc.tensor.matmul(
                        btp[:, :cn],
                        lhsT=boxm[:, q, :],
                        rhs=s16[b][:, ds(t0 + q, cn)],
                        start=(q == 0),
                        stop=(q == 2),
                    )
                # scale on DVE: the scalar queue's sign chunks would delay
                # this, and the single psb bank WARs the next seg's matmuls
                # against it
                nc.vector.tensor_scalar_mul(
                    betaCol[b][:, ds(t0, cn)], btp[:, :cn], 1.0 / D_DIM
                )

            def emit_mm_pair(b, p):
                ntile = 2 if p < N_PAIRS - 1 else 1
                ps = psump.tile([128, 2, F], F32, tag="ps", name=f"ps{b}_{p}")
                base = (2 * p) * 128
                nmm = NPAIR * ntile
                k = 0
                for j in range(ntile):
                    for i, (da, db) in enumerate(PAIR_OFF):
                        nc.tensor.matmul(
                            ps[:, j, :],
                            lhsT=_pair_view(
                                xq[b][:, :], base + j * 128 + da, base + j * 128 + db, 128
                            ),
                            rhs=w8[:, ds(2 * i, 2), :],
                            start=(k == 0),
                            stop=(k == nmm - 1),
                            perf_mode=mybir.MatmulPerfMode.DoubleRow,
                        )
                        k += 1
                return ps

            o2cur = [None] * B_PER_CORE

            def emit_drain_bank(b, p, ps_tiles):
                nonlocal nbank, nstore
                t0 = 2 * p
                ntile = 2 if p < N_PAIRS - 1 else 1
                ps = ps_tiles[p]
                if t0 % STG == 0:
                    o2cur[b] = o2p.tile(
                        [128, STG, F], BF16, tag="o2", name=f"o2_{b}_{t0}"
                    )
                o2t = o2cur[b]
                sl = t0 % STG
                if ntile == 2 and nbank % 3 != 2:
                    bc = (
                        betaCol[b][:, ds(t0, 2)]
                        .unsqueeze(2)
                        .broadcast_to([128, 2, F])
                    )
                    nc.vector.tensor_mul(o2t[:, ds(sl, 2), :], ps[:, :, :], bc)
                else:
                    for j in range(ntile):
                        nc.scalar.mul(
                            o2t[:, sl + j, :],
                            ps[:, j, :],
                            betaCol[b][:, ds(t0 + j, 1)],
                        )
                nbank += 1
                t_last = t0 + ntile - 1
                if t_last % STG == STG - 1 or t_last == N_TILES - 1:
                    n = t_last % STG + 1
                    g0 = t_last - n + 1
                    nc.sync.dma_start(
                        o_d[ds((b * N_TILES + g0) * 128, n * 128), :].rearrange(
                            "(t p) f -> p t f", t=n
                        ),
                        o2t[:, :n, :],
                    )
                    nstore += 1

            # ---------- interleaved two-image pipeline ----------
            ps_tiles = [[] for _ in range(B_PER_CORE)]
            rsl = [0] * B_PER_CORE      # reduce slices emitted
            bq = [0] * B_PER_CORE       # next beta seg
            betac = [0] * B_PER_CORE    # tiles covered by betaCol
            mm_p = [0] * B_PER_CORE     # next psum bank (px-tile pair)
            dr_p = [0] * B_PER_CORE     # next bank to drain
            pcov_v = [0] * B_PER_CORE
            for c in range(len(XT_CH)):
                for b in range(B_PER_CORE):
                    c0, npix = XT_CH[c]
                    emit_xt_chunk(b, c0, npix)
                    pcov_v[b] = c0 + npix
                if c == 0:
                    emit_xpm_pieces()
                progressed = True
                while progressed:
                    progressed = False
                    for b in range(B_PER_CORE):
                        # demand-driven beta: stay ~5 banks ahead of drains,
                        # pulling reduce slices just-in-time so the DVE queue
                        # never front-loads reduces ahead of pending drains
                        while bq[b] < len(BSEGS) and betac[b] < min(
                            2 * dr_p[b] + 14, N_TILES
                        ):
                            t0, cn = BSEGS[bq[b]]
                            need = t0 + cn + 3
                            while (
                                rsl[b] < N_RSL
                                and RSL * rsl[b] + 2 * (rsl[b] == N_RSL) < need
                            ):
                                emit_reduce_slice(b, rsl[b])
                                rsl[b] += 1
                            emit_beta_seg(b, t0, cn)
                            betac[b] = t0 + cn
                            bq[b] += 1
                            progressed = True
                        # mid-kernel: drip remaining reduce slices so late
                        # beta segs never wait on them (data resident by now)
                        if rsl[b] < N_RSL and dr_p[b] >= 8:
                            emit_reduce_slice(b, rsl[b])
                            rsl[b] += 1
                            progressed = True
                        if mm_p[b] < N_PAIRS:
                            p = mm_p[b]
                            last_t = min(2 * p + 1, N_TILES - 1)
                            # planes reach px base+353 (max tap offset 226+127)
                            need_px = 128 * last_t + 354
                            if need_px <= pcov_v[b]:
                                ps_tiles[b].append(emit_mm_pair(b, p))
                                mm_p[b] += 1
                                progressed = True
                        while dr_p[b] < mm_p[b] and min(
                            2 * dr_p[b] + 2, N_TILES
                        ) <= betac[b]:
                            emit_drain_bank(b, dr_p[b], ps_tiles[b])
                            dr_p[b] += 1
                            progressed = True
            assert rsl == [N_RSL] * B_PER_CORE, rsl
            assert bq == [len(BSEGS)] * B_PER_CORE, bq
            assert mm_p == [N_PAIRS] * B_PER_CORE, mm_p
            assert dr_p == [N_PAIRS] * B_PER_CORE, dr_p

    nc.compile()
    return nc


_NC = None


def _get_nc():
    global _NC
    if _NC is None:
        _NC = build_nc()
    return _NC


def _quantize_weights(kernels, alphas):
    """Fold estimators, then per-channel-scale fp8 e4m3 quantization."""
    sgn = np.where(kernels >= 0, 1.0, -1.0).astype(np.float32)  # [E,3,3,128,256]
    W = np.einsum("ehwcf,ef->hwcf", sgn, alphas.astype(np.float32))  # [3,3,128,256]
    # scale search on the single-term taps; tap (2,2) is two-term (near exact)
    Wf = np.stack([W[kh, kw] for kh, kw in SLOT_TAPS]).reshape(8 * CIN, F)
    scales = np.geomspace(6.0, 100.0, 385).astype(np.float32)
    q = (Wf[None, :, :] * scales[:, None, None]).astype(ml_dtypes.float8_e4m3fn)
    err = ((q.astype(np.float32) / scales[:, None, None] - Wf[None]) ** 2).sum(axis=1)
    s = scales[np.argmin(err, axis=0)]  # [F]
    Wq = (W * s).astype(ml_dtypes.float8_e4m3fn)  # [3,3,128,256]
    Wres = W * s - Wq.astype(np.float32)
    Wq2 = Wres.astype(ml_dtypes.float8_e4m3fn)  # second term for tap (2,2)
    # slot pairs (2i, 2i+1) follow PAIR_OFF via SLOT_TAPS; tap (2,2) two-term
    w8 = np.zeros((CIN, 2 * NPAIR, F), ml_dtypes.float8_e4m3fn)
    for j, (kh, kw) in enumerate(SLOT_TAPS):
        w8[:, j, :] = Wq[kh, kw]
    w8[:, 8, :] = Wq[2, 2]
    w8[:, 9, :] = Wq2[2, 2]
    sinv = (1.0 / s).astype(np.float32)  # [F]; applied on host after gather
    return np.ascontiguousarray(w8.reshape(CIN, 2 * NPAIR * F)), sinv


def _in_maps(x, kernels, alphas):
    x = np.asarray(x, np.float32)
    kernels = np.asarray(kernels, np.float32)
    alphas = np.asarray(alphas, np.float32)
    w8, sinv = _quantize_weights(kernels, alphas)
    boxm = _box_matrices()

    xb = x.astype(ml_dtypes.bfloat16)  # sign-exact; |x| sums lose <0.1%
    xs = xb.reshape(N_CORES, B_PER_CORE, IMG_PIX, CIN)
    maps = []
    for c in range(N_CORES):
        xT = np.zeros((CIN, B_PER_CORE * IMG_PAD), ml_dtypes.bfloat16)
        for b in range(B_PER_CORE):
            xT[:, b * IMG_PAD : b * IMG_PAD + IMG_PIX] = xs[c, b].T
        xpm = np.ascontiguousarray(
            xs[c].reshape(B_PER_CORE, S_TILES, 128, CIN)
            .transpose(2, 0, 1, 3)
            .reshape(128, B_PER_CORE * S_TILES, CIN)
        )
        maps.append(
            {
                "xT": np.ascontiguousarray(xT),
                "xpm": xpm,
                "w8": w8,
                "boxm": boxm,
            }
        )
    return maps, sinv


def _gather(results, sinv):
    outs = []
    sv = sinv.reshape(1, 1, 1, F)
    for c in range(N_CORES):
        o = np.asarray(results[c]["out"]).astype(np.float32)  # [24832, 256]
        o = o.reshape(B_PER_CORE, STAGE_PIX, F)[:, :OUT_PIX]
        o = o.reshape(B_PER_CORE, OUT_ROWS, W_IMG, F)[:, :, :OUT_ROWS] * sv
        outs.append(o)
    return np.ascontiguousarray(np.concatenate(outs, axis=0))


def kernel(x, kernels, alphas):
    nc = _get_nc()
    maps, sinv = _in_maps(x, kernels, alphas)
    res = run_bass_kernel_spmd(nc, maps, core_ids=list(range(N_CORES)))
    return _gather(res.results, sinv)


def _install_profile_hook():
    """The agent image's antenv lacks axon_hooks; recreate it so
    run_bass_kernel_spmd(trace=True) can NTFF-profile via libaxon_pjrt.so."""
    import types

    import antenv

    if "antenv.axon_hooks" in sys.modules:
        return
    mod = types.ModuleType("antenv.axon_hooks")
    holder = {}
    mod.set_axon_ntff_profile_hook = lambda h: holder.__setitem__("h", h)
    mod.get_axon_ntff_profile_hook = lambda: holder.get("h")
    sys.modules["antenv.axon_hooks"] = mod
    antenv.axon_hooks = mod

    from trn_agent_boot.trn_boot import _ntff_profile_via_ctypes

    hook = _ntff_profile_via_ctypes("/opt/axon/libaxon_pjrt.so")
    mod.set_axon_ntff_profile_hook(hook)

    # upload_artifacts wants a cloud bucket; keep everything local instead.
    import concourse.bass_utils as bu

    bu.upload_artifacts = lambda tmpdir: tmpdir


def run_profiled(x, kernels, alphas, tmpdir=None):
    """Returns (output, exec_time_ns, profile_json_path)."""
    _install_profile_hook()
    nc = _get_nc()
    maps, sinv = _in_maps(x, kernels, alphas)
    res = run_bass_kernel_spmd(
        nc,
        maps,
        core_ids=list(range(N_CORES)),
        trace=True,
        tmpdir=tmpdir,
    )
    return _gather(res.results, sinv), res.exec_time_ns, res.profile_json
